# revision 45
# baseline (speedup 1.0000x reference)
"""Multi-head attention (B=2, N=2048, D=1024, H=16) on 8 Trainium2 cores.

Sharding: data-parallel over batch (2) x tensor-parallel over head groups (4).
Core c handles batch c//4, heads 4*(c%4) .. 4*(c%4)+3.

Per-core kernel, all matmul operands bf16 (rate 1.0 cycles/row at any width;
fp8 DoubleRow would halve PE time but its ~4-6% elementwise noise fails the
2e-2 max-err gate -- measured in numpy).  Every matmul uses a uniform
(128,128) PE tile at base partition 0: alternating tile_position row bases
(0 <-> 64) between consecutive bf16 matmuls crashes the runtime, so S uses a
full-128 contraction against a per-head zero-padded qT instead of
64-partition tiles.

  front:   kT = (Wk^T x^T) + bias        (channels on partitions, packed)
           qTz = (Wq^T x^T) + bias       (per-head slots, off-head rows = 0)
           v   = (x Wv) + bv             (tokens on partitions, [v|1] cols)
  per query-tile of 256 (ACT-exp paces at ~1.04us/key-ptile, 128 exp ops
  of [128,1024] = the ~133us Activation floor):
    per key-ptile kt2 (128 keys):
      S^T[:,h,:] = kT_pair^T qTz_h       (zeros kill the cross-head terms)
      P^T = exp(SCALE * S^T)             (one ACT op over all 4 heads)
      O[q,s,h*65:+65] += P^T_s^T [v_h|1] (65-col streams, queries on
                                          partitions; col 64 = softmax sum)
    PV/normalize are deferred behind a deep pt pool (ACT never waits on the
    qt0 projection hump); O /= sums via DVE per-partition scalars;
    transpose back via DMA crossbar (PE+identity for the last qt);
    out[tokens] = sum_ct oT_ct^T @ Wo_ct   (chains of 2, K=128 each)
Host: out[b] = sum of the 4 group partials + b_o.

Cost-model timeline: ~177us (baseline 296us); PE busy ~141us, ACT ~136us.
"""

import sys

sys.path.insert(0, "/opt/trn_rl_repo")

import numpy as np

B, N, D, H = 2, 2048, 1024, 16
SUB = D // H  # 64
GROUPS = 4  # tensor-parallel head groups
NH = H // GROUPS  # 4 local heads per core
CH = NH * SUB  # 256 local channels
NCORES = 8
QB = 256  # query tile
NQT = N // QB  # 8
KT = D // 128  # contraction ptiles
TOKT = N // 128  # token/key ptiles
SCALE = SUB ** -0.5
WARMUP = True
SPLIT_EXP = True
DO_TAIL = True
DO_PV = True
DO_V = True
DO_QK = True


def build_nc(name="mha"):
    import concourse.mybir as mybir
    from concourse import bacc
    from concourse.tile import TileContext

    f32 = mybir.dt.float32
    bf16 = mybir.dt.bfloat16
    Exp = mybir.ActivationFunctionType.Exp

    nc = bacc.Bacc(None, name=name)
    # host-packed, partition-major layouts (see make_in_maps)
    xh = nc.dram_tensor("xh", [128, KT, N], bf16, kind="ExternalInput")
    wq = nc.dram_tensor("wq", [128, KT, CH], bf16, kind="ExternalInput")
    wk = nc.dram_tensor("wk", [128, KT, CH], bf16, kind="ExternalInput")
    wv = nc.dram_tensor("wv", [128, KT, CH], bf16, kind="ExternalInput")
    wvb = nc.dram_tensor("wvb", [128, NH, SUB], bf16, kind="ExternalInput")
    wo = nc.dram_tensor("wo", [128, 2, D], bf16, kind="ExternalInput")
    bqk = nc.dram_tensor("bqk", [128, 4], f32, kind="ExternalInput")
    ones_d = nc.dram_tensor("ones", [128, 128], bf16, kind="ExternalInput")
    ident_d = nc.dram_tensor("ident", [128, 128], bf16, kind="ExternalInput")
    out = nc.dram_tensor("out", [N, D], bf16, kind="ExternalOutput")

    with TileContext(nc) as tc:
        with tc.tile_pool(name="persist", bufs=1) as pp:
            xt = pp.tile([128, KT, N], bf16)
            wq_sb = pp.tile([128, KT, CH], bf16)
            wk_sb = pp.tile([128, KT, CH], bf16)
            wv_sb = pp.tile([128, KT, CH], bf16)
            wvb_sb = pp.tile([128, NH, SUB], bf16)
            wo_sb = pp.tile([128, 2, D], bf16)
            qT_sb = pp.tile([128, NH, N], bf16)
            kT_sb = pp.tile([128, 2, N], bf16)
            v_sb = pp.tile([128, TOKT, NH, 65], bf16)
            oT_sb = pp.tile([128, 2, N], bf16)
            bqk_sb = pp.tile([128, 4], f32)
            ident_sb = pp.tile([128, 128], bf16)

            # DMA issue order: earliest-needed first. x comes in 4 token
            # quarters so the first projections can start at ~4.5us.
            nc.sync.dma_start(wk_sb[:, 0:4, :], wk[:, 0:4, :])
            nc.sync.dma_start(bqk_sb[:], bqk[:])
            nc.sync.dma_start(xt[:, 0:2, 0:256], xh[:, 0:2, 0:256])
            nc.sync.dma_start(xt[:, 2:4, 0:256], xh[:, 2:4, 0:256])
            nc.sync.dma_start(wq_sb[:, 0:4, :], wq[:, 0:4, :])
            nc.sync.dma_start(wk_sb[:, 4:8, :], wk[:, 4:8, :])
            nc.sync.dma_start(xt[:, 4:6, 0:256], xh[:, 4:6, 0:256])
            nc.sync.dma_start(xt[:, 6:8, 0:256], xh[:, 6:8, 0:256])
            nc.sync.dma_start(wq_sb[:, 4:8, :], wq[:, 4:8, :])
            nc.sync.dma_start(wv_sb[:], wv[:])
            nc.sync.dma_start(wvb_sb[:], wvb[:])
            nc.sync.dma_start(xt[:, :, 256:512], xh[:, :, 256:512])
            nc.sync.dma_start(xt[:, :, 512:768], xh[:, :, 512:768])
            nc.sync.dma_start(xt[:, :, 768:1024], xh[:, :, 768:1024])
            nc.sync.dma_start(wo_sb[:], wo[:])
            nc.sync.dma_start(ident_sb[:], ident_d[:])
            for sl in range(4, 8):
                nc.sync.dma_start(
                    xt[:, :, sl * 256 : (sl + 1) * 256],
                    xh[:, :, sl * 256 : (sl + 1) * 256],
                )

            with tc.tile_pool(name="stp", bufs=2, space="PSUM") as stp, \
                 tc.tile_pool(name="op_", bufs=1, space="PSUM") as op_, \
                 tc.tile_pool(name="aux", bufs=2, space="PSUM") as aux, \
                 tc.tile_pool(name="ptp", bufs=26) as ptp, \
                 tc.tile_pool(name="osb", bufs=3) as osb, \
                 tc.tile_pool(name="rcpp", bufs=2) as rcpp, \
                 tc.tile_pool(name="stg", bufs=2) as stgp:

                def qk_proj(dst, wt, bcol, mt, s, per_head=False):
                    """dst[:, mt, 256s:+256] = (W^T x^T)[128ch, 256tok] + bias.

                    per_head (for the zero-padded qT): each head's 64
                    channels land in its own head-slot, same partitions."""
                    ps = aux.tile([128, 512], f32, name="ps", tag="aux")
                    for kt in range(KT):
                        nc.tensor.matmul(
                            ps[:, 0:QB],
                            lhsT=wt[:, kt, mt * 128 : (mt + 1) * 128],
                            rhs=xt[:, kt, s * QB : (s + 1) * QB],
                            start=(kt == 0),
                            stop=(kt == KT - 1),
                        )
                    if per_head:
                        for hh in range(2):
                            a = 64 * hh
                            nc.vector.tensor_scalar_add(
                                dst[a : a + 64, 2 * mt + hh, s * QB : (s + 1) * QB],
                                ps[a : a + 64, 0:QB],
                                bqk_sb[a : a + 64, bcol + mt : bcol + mt + 1],
                            )
                    else:
                        nc.vector.tensor_scalar_add(
                            dst[:, mt, s * QB : (s + 1) * QB],
                            ps[:, 0:QB],
                            bqk_sb[:, bcol + mt : bcol + mt + 1],
                        )

                def v_proj(tt):
                    """v_sb[:, tt, :, 0:64] = ([x;1] @ [Wv;bv])[128tok, 256ch]."""
                    ps = aux.tile([128, 8, 64], f32, name="psv", tag="aux")
                    for kt in range(KT):
                        nc.tensor.matmul(
                            ps[:, 0:NH, :],
                            lhsT=xt[:, kt, tt * 128 : (tt + 1) * 128],
                            rhs=wv_sb[:, kt, :],
                            start=(kt == 0),
                            stop=(kt == KT - 1),
                        )
                    nc.vector.tensor_add(
                        v_sb[:, tt, :, 0:64], ps[:, 0:NH, :], wvb_sb[:]
                    )

                def transpose_pair(osb_t, qt, s):
                    """oT_sb[:, blk, qt*256+128s:+128] = osb_t[:, blk, :]^T.

                    DMA crossbar (off the PE/DVE path) in steady state; PE
                    transpose for the last qt where DMA latency would land
                    on the drain tail."""
                    tr = aux.tile([128, 2, 128], bf16, name="tr", tag="aux")
                    for blk in range(2):
                        nc.tensor.transpose(tr[:, blk, :], osb_t[:, 128 * blk : 128 * (blk + 1)], ident_sb[:])
                    for blk in range(2):
                        nc.vector.tensor_copy(
                            oT_sb[:, blk, qt * QB + 128 * s : qt * QB + 128 * (s + 1)],
                            tr[:, blk, :],
                        )

                def outproj(tt):
                    """out[tt*128:+128, :] = sum_ct oT_ct^T @ Wo_ct.

                    Stage copies go through the idle Pool engine in steady
                    state (keeps DVE free for the recip/normalize chain);
                    the last qt alternates DVE/ACT so the tail copies run
                    in parallel."""
                    tail = tt >= 2 * (NQT - 1)
                    stg = None
                    for nt in range(2):
                        ps = aux.tile([128, 512], f32, name="ops", tag="aux")
                        for ct in range(2):
                            nc.tensor.matmul(
                                ps[:],
                                lhsT=oT_sb[:, ct, tt * 128 : (tt + 1) * 128],
                                rhs=wo_sb[:, ct, nt * 512 : (nt + 1) * 512],
                                start=(ct == 0),
                                stop=(ct == 1),
                            )
                        if not tail:
                            stg = stgp.tile([128, 512], bf16, name="stg", tag="stg")
                            nc.vector.tensor_copy(stg[:], ps[:])
                            nc.sync.dma_start(
                                out[
                                    tt * 128 : (tt + 1) * 128,
                                    nt * 512 : (nt + 1) * 512,
                                ],
                                stg[:],
                            )
                            continue
                        # tail: copies run on DVE and ACT in parallel, one
                        # batched store per token tile
                        if nt == 0:
                            stg = stgp.tile([128, 1024], bf16, name="stg2", tag="st2")
                            nc.vector.tensor_copy(stg[:, 0:512], ps[:])
                        else:
                            nc.scalar.copy(stg[:, 512:1024], ps[:])
                            nc.sync.dma_start(
                                out[tt * 128 : (tt + 1) * 128, :], stg[:]
                            )

                def emit(item):
                    kind = item[0]
                    if kind == "q":
                        qk_proj(qT_sb, wq_sb, 0, item[1], item[2])
                    elif kind == "k":
                        qk_proj(kT_sb, wk_sb, 2, item[1], item[2])
                    elif kind == "v":
                        v_proj(item[1])
                    elif kind == "o":
                        outproj(item[1])

                from collections import deque

                pending = deque()

                # ones column of [v|1] (fused softmax denominators)
                nc.vector.memset(v_sb[:, :, :, 64:65], 1.0)
                # zero the off-head partitions of the per-head padded qT so
                # S can contract over all 128 partitions at tile base 0
                # (on the idle gpsimd engine, off the DVE critical path;
                # qt0's columns first so the first S never waits)
                for h in range(NH):
                    a = 64 * ((h + 1) % 2)
                    nc.gpsimd.memset(qT_sb[a : a + 64, h, 0:QB], 0.0)
                for h in range(NH):
                    a = 64 * ((h + 1) % 2)
                    nc.gpsimd.memset(qT_sb[a : a + 64, h, QB:N], 0.0)

                # PE p-state warmup: keep the PE continuously busy from t~0 so
                # it reaches full clock (3us ramp) before the real work lands.
                warm = pp.tile([128, 256], bf16)
                nc.vector.memset(warm[:], 0.0)
                wps = aux.tile([128, 512], f32, name="wps", tag="aux")
                for i in range(11 if WARMUP else 0):
                    nc.tensor.matmul(
                        wps[:, 0:256],
                        lhsT=warm[:, 0:128],
                        rhs=warm[:, :],
                        start=True,
                        stop=True,
                        skip_group_check=True,
                    )

                # front: projections for tokens 0-255 (x slice 0 only), in
                # DMA-arrival order; v goes after the first S/exp so the late
                # wv DMA never blocks the ACT stream start
                qk_proj(kT_sb, wk_sb, 2, 0, 0)
                qk_proj(qT_sb, wq_sb, 0, 0, 0, per_head=True)
                qk_proj(kT_sb, wk_sb, 2, 1, 0)
                qk_proj(qT_sb, wq_sb, 0, 1, 0, per_head=True)

                def s_exp(qt, kt2, split=False):
                    """S^T + exp for (qt, kt2); returns the pt tile.

                    split=True runs exp per head-pair so the first tiles
                    start as soon as the mt0 projections land."""
                    st = stp.tile([128, NH, QB], f32, name="st", tag="st")
                    pt = ptp.tile([128, NH, QB], bf16, name="pt", tag="pt")
                    for half in range(2):
                        for hh in range(2):
                            h = 2 * half + hh
                            nc.tensor.matmul(
                                st[:, h, :],
                                lhsT=kT_sb[
                                    :, h // 2, kt2 * 128 : (kt2 + 1) * 128
                                ],
                                rhs=qT_sb[:, h, qt * QB : (qt + 1) * QB],
                                start=True,  # overwrite; st is read-only after
                                stop=True,
                                skip_group_check=True,
                            )
                        if split and half == 0:
                            nc.scalar.activation(
                                pt[:, 0:2, :], st[:, 0:2, :], Exp, scale=SCALE
                            )
                    if split:
                        nc.scalar.activation(
                            pt[:, 2:4, :], st[:, 2:4, :], Exp, scale=SCALE
                        )
                    else:
                        nc.scalar.activation(pt[:], st[:], Exp, scale=SCALE)
                    return pt

                def pv(pt, kt2, o_ps):
                    first, last = kt2 == 0, kt2 == TOKT - 1
                    for s in range(2):
                        for h in range(NH):
                            nc.tensor.matmul(
                                o_ps[:, s, 65 * h : 65 * h + 65],
                                lhsT=pt[:, h, 128 * s : 128 * (s + 1)],
                                rhs=v_sb[:, kt2, h, :],
                                start=(first and h == 0),  # opener per s-bank
                                stop=last,
                                skip_group_check=True,
                            )

                def norm(qt, o_ps):
                    rcp = rcpp.tile([128, 8], f32, name="rcp", tag="rcp")
                    for s in range(2):
                        nc.vector.reciprocal(
                            rcp[:, 4 * s : 4 * s + 4], o_ps[:, s, 64:324:65]
                        )
                        osb_t = osb.tile([128, 256], bf16, name="osb", tag="osb")
                        for h in range(NH):
                            nc.vector.tensor_scalar_mul(
                                osb_t[:, 64 * h : 64 * h + 64],
                                o_ps[:, s, 65 * h : 65 * h + 64],
                                rcp[:, 4 * s + h : 4 * s + h + 1],
                            )
                        # crossbar transpose issues immediately (SP engine);
                        # the out-proj chains stream through later ACT windows
                        if DO_TAIL:
                            transpose_pair(osb_t, qt, s)
                            if qt == NQT - 1:
                                outproj(2 * qt + s)
                            else:
                                pending.append((BODY[0] + 3, ("o", 2 * qt + s)))

                # Deferred PV/normalize stream: ACT runs ahead of PV by up to
                # LAG tiles (deep pt pool), so the heavy qt0 projection
                # backlog never stalls the exp stream; the lag drains over
                # the last two qts to keep the tail short.
                LAG0 = 20
                HUMP = 3 * TOKT
                END = (NQT - 1) * TOKT
                pv_q = deque()
                BODY = [0]
                pt_next = s_exp(0, 0, split=SPLIT_EXP)
                v_proj(0)
                v_proj(1)
                for qt in range(NQT):
                    o_ps = op_.tile([128, 2, 512], f32, name="o_ps", tag="o")
                    for kt2 in range(TOKT):
                        body = qt * TOKT + kt2
                        BODY[0] = body
                        last = kt2 == TOKT - 1
                        pv_q.append(
                            lambda pt=pt_next, kt2=kt2, o=o_ps: pv(pt, kt2, o)
                        )
                        # software pipeline: S/exp of the NEXT tile go ahead of
                        # this tile's PV so the ACT stream never waits on PE
                        if not last:
                            pt_next = s_exp(qt, kt2 + 1, split=(SPLIT_EXP and qt == 0 and kt2 == 0))
                        elif qt + 1 < NQT:
                            pt_next = s_exp(qt + 1, 0)
                        # high-priority streamed projections; kT is on the
                        # exp critical path, v only feeds the (deeply lagged)
                        # PV stream so it spreads into qt1's idle PE windows
                        if qt == 0:
                            s_next = kt2 // 2 + 1
                            if kt2 % 2 == 0 and s_next < 8:
                                qk_proj(kT_sb, wk_sb, 2, 0, s_next)
                                qk_proj(kT_sb, wk_sb, 2, 1, s_next)
                        if DO_V and 2 <= body - 10 < TOKT:
                            v_proj(body - 10)
                        if kt2 == 8 and qt + 1 < NQT:
                            qk_proj(qT_sb, wq_sb, 0, 0, qt + 1, per_head=True)
                        if kt2 == 9 and qt + 1 < NQT:
                            qk_proj(qT_sb, wq_sb, 0, 1, qt + 1, per_head=True)
                        if kt2 >= 2 and pending and body >= pending[0][0]:
                            emit(pending.popleft()[1])
                            if qt == NQT - 1 and pending and body >= pending[0][0]:
                                emit(pending.popleft()[1])
                        lag = max(14, LAG0 - max(0, body - HUMP) // 2)
                        if body >= END:
                            lag = 1
                        while DO_PV and len(pv_q) > lag:
                            pv_q.popleft()()
                    pv_q.append(lambda qt=qt, o=o_ps: norm(qt, o))
                while DO_PV and pv_q:
                    pv_q.popleft()()
                while pending:
                    emit(pending.popleft()[1])
    nc.finalize()
    return nc


def make_in_maps(x, W_qkv, b_qkv, W_o):
    """Shard full inputs into per-core input maps (core c: batch c//4, group c%4)."""
    import ml_dtypes

    bf16 = ml_dtypes.bfloat16
    x = np.asarray(x, dtype=np.float32)
    W_qkv = np.asarray(W_qkv, dtype=np.float32)
    b_qkv = np.asarray(b_qkv, dtype=np.float32)
    W_o = np.asarray(W_o, dtype=np.float32)

    def pack_w(w):  # [1024, CH] -> [128, KT, CH] partition-major
        return np.ascontiguousarray(
            w.reshape(KT, 128, CH).transpose(1, 0, 2).astype(bf16)
        )

    in_maps = []
    for c in range(NCORES):
        b, g = divmod(c, GROUPS)
        cols = slice(CH * g, CH * (g + 1))
        bq = b_qkv[0 * D : 1 * D][cols]
        bk = b_qkv[1 * D : 2 * D][cols]
        bqk = np.stack(
            [bq[0:128], bq[128:256], bk[0:128], bk[128:256]], axis=1
        ).astype(np.float32)
        xh = (
            x[b].T.reshape(KT, 128, N).transpose(1, 0, 2).astype(bf16)
        )  # [128, KT, N]
        m = {
            "xh": np.ascontiguousarray(xh),
            "wq": pack_w(W_qkv[:, 0 * D : 1 * D][:, cols]),
            "wk": pack_w(W_qkv[:, 1 * D : 2 * D][:, cols]),
            "wv": pack_w(W_qkv[:, 2 * D : 3 * D][:, cols]),
            "wvb": np.ascontiguousarray(
                np.broadcast_to(
                    b_qkv[2 * D : 3 * D][cols].reshape(NH, SUB), (128, NH, SUB)
                ).astype(bf16)
            ),
            "wo": np.ascontiguousarray(
                W_o[cols, :].reshape(2, 128, D).transpose(1, 0, 2).astype(bf16)
            ),
            "bqk": np.ascontiguousarray(bqk),
            "ones": np.ones((128, 128), dtype=bf16),
            "ident": np.eye(128, dtype=bf16),
        }
        in_maps.append(m)
    return in_maps


_NC = None


def get_nc():
    global _NC
    if _NC is None:
        _NC = build_nc()
    return _NC


def kernel(x, W_qkv, b_qkv, W_o, b_o):
    from concourse import bass_utils

    b_o = np.asarray(b_o, dtype=np.float32)
    in_maps = make_in_maps(x, W_qkv, b_qkv, W_o)
    res = bass_utils.run_bass_kernel_spmd(get_nc(), in_maps, core_ids=list(range(NCORES)))
    out = np.empty((B, N, D), dtype=np.float32)
    for b in range(B):
        acc = res.results[4 * b]["out"].astype(np.float32)
        for g in range(1, GROUPS):
            acc += res.results[4 * b + g]["out"].astype(np.float32)
        out[b] = acc + b_o
    return out


# revision 46
# speedup vs baseline: 1.0172x; 1.0172x over previous
"""Multi-head attention (B=2, N=2048, D=1024, H=16) on 8 Trainium2 cores.

Sharding: data-parallel over batch (2) x tensor-parallel over head groups (4).
Core c handles batch c//4, heads 4*(c%4) .. 4*(c%4)+3.

Per-core kernel, all matmul operands bf16 (rate 1.0 cycles/row at any width;
fp8 DoubleRow would halve PE time but its ~4-6% elementwise noise fails the
2e-2 max-err gate -- measured in numpy).  Every matmul uses a uniform
(128,128) PE tile at base partition 0: alternating tile_position row bases
(0 <-> 64) between consecutive bf16 matmuls crashes the runtime, so S uses a
full-128 contraction against a per-head zero-padded qT instead of
64-partition tiles.

  front:   kT = (Wk^T x^T) + bias        (channels on partitions, packed)
           qTz = (Wq^T x^T) + bias       (per-head slots, off-head rows = 0)
           v   = (x Wv) + bv             (tokens on partitions, [v|1] cols)
  per query-tile of 256 (ACT-exp paces at ~1.04us/key-ptile, 128 exp ops
  of [128,1024] = the ~133us Activation floor):
    per key-ptile kt2 (128 keys):
      S^T[:,h,:] = kT_pair^T qTz_h       (zeros kill the cross-head terms)
      P^T = exp(SCALE * S^T)             (one ACT op over all 4 heads)
      O[q,s,h*65:+65] += P^T_s^T [v_h|1] (65-col streams, queries on
                                          partitions; col 64 = softmax sum)
    PV/normalize are deferred behind a deep pt pool (ACT never waits on the
    qt0 projection hump); O /= sums via DVE per-partition scalars;
    transpose back via DMA crossbar (PE+identity for the last qt);
    out[tokens] = sum_ct oT_ct^T @ Wo_ct   (chains of 2, K=128 each)
Host: out[b] = sum of the 4 group partials + b_o.

Cost-model timeline: ~177us (baseline 296us); PE busy ~141us, ACT ~136us.
"""

import sys

sys.path.insert(0, "/opt/trn_rl_repo")

import numpy as np

B, N, D, H = 2, 2048, 1024, 16
SUB = D // H  # 64
GROUPS = 4  # tensor-parallel head groups
NH = H // GROUPS  # 4 local heads per core
CH = NH * SUB  # 256 local channels
NCORES = 8
QB = 256  # query tile
NQT = N // QB  # 8
KT = D // 128  # contraction ptiles
TOKT = N // 128  # token/key ptiles
SCALE = SUB ** -0.5
WARMUP = True
SPLIT_EXP = True
DO_TAIL = True
DO_PV = True
DO_V = True
DO_QK = True


def build_nc(name="mha"):
    import concourse.mybir as mybir
    from concourse import bacc
    from concourse.tile import TileContext

    f32 = mybir.dt.float32
    bf16 = mybir.dt.bfloat16
    Exp = mybir.ActivationFunctionType.Exp

    nc = bacc.Bacc(None, name=name)
    # host-packed, partition-major layouts (see make_in_maps)
    xh = nc.dram_tensor("xh", [128, KT, N], bf16, kind="ExternalInput")
    wq = nc.dram_tensor("wq", [128, KT, CH], bf16, kind="ExternalInput")
    wk = nc.dram_tensor("wk", [128, KT, CH], bf16, kind="ExternalInput")
    wv = nc.dram_tensor("wv", [128, KT, CH], bf16, kind="ExternalInput")
    wvb = nc.dram_tensor("wvb", [128, NH, SUB], bf16, kind="ExternalInput")
    wo = nc.dram_tensor("wo", [128, 2, D], bf16, kind="ExternalInput")
    bqk = nc.dram_tensor("bqk", [128, 4], f32, kind="ExternalInput")
    ones_d = nc.dram_tensor("ones", [128, 128], bf16, kind="ExternalInput")
    ident_d = nc.dram_tensor("ident", [128, 128], bf16, kind="ExternalInput")
    out = nc.dram_tensor("out", [N, D], bf16, kind="ExternalOutput")

    with TileContext(nc) as tc:
        with tc.tile_pool(name="persist", bufs=1) as pp:
            xt = pp.tile([128, KT, N], bf16)
            wq_sb = pp.tile([128, KT, CH], bf16)
            wk_sb = pp.tile([128, KT, CH], bf16)
            wv_sb = pp.tile([128, KT, CH], bf16)
            wvb_sb = pp.tile([128, NH, SUB], bf16)
            wo_sb = pp.tile([128, 2, D], bf16)
            qT_sb = pp.tile([128, NH, N], bf16)
            kT_sb = pp.tile([128, 2, N], bf16)
            v_sb = pp.tile([128, TOKT, NH, 65], bf16)
            oT_sb = pp.tile([128, 2, N], bf16)
            bqk_sb = pp.tile([128, 4], f32)
            ident_sb = pp.tile([128, 128], bf16)

            # DMA issue order: earliest-needed first. x comes in 4 token
            # quarters so the first projections can start at ~4.5us.
            nc.sync.dma_start(wk_sb[:, 0:4, :], wk[:, 0:4, :])
            nc.sync.dma_start(bqk_sb[:], bqk[:])
            nc.sync.dma_start(xt[:, 0:2, 0:256], xh[:, 0:2, 0:256])
            nc.sync.dma_start(xt[:, 2:4, 0:256], xh[:, 2:4, 0:256])
            nc.sync.dma_start(wq_sb[:, 0:4, :], wq[:, 0:4, :])
            nc.sync.dma_start(wk_sb[:, 4:8, :], wk[:, 4:8, :])
            nc.sync.dma_start(xt[:, 4:6, 0:256], xh[:, 4:6, 0:256])
            nc.sync.dma_start(xt[:, 6:8, 0:256], xh[:, 6:8, 0:256])
            nc.sync.dma_start(wq_sb[:, 4:8, :], wq[:, 4:8, :])
            nc.sync.dma_start(wv_sb[:], wv[:])
            nc.sync.dma_start(wvb_sb[:], wvb[:])
            nc.sync.dma_start(xt[:, :, 256:512], xh[:, :, 256:512])
            nc.sync.dma_start(xt[:, :, 512:768], xh[:, :, 512:768])
            nc.sync.dma_start(xt[:, :, 768:1024], xh[:, :, 768:1024])
            nc.sync.dma_start(wo_sb[:], wo[:])
            nc.sync.dma_start(ident_sb[:], ident_d[:])
            for sl in range(4, 8):
                nc.sync.dma_start(
                    xt[:, :, sl * 256 : (sl + 1) * 256],
                    xh[:, :, sl * 256 : (sl + 1) * 256],
                )

            with tc.tile_pool(name="stp", bufs=2, space="PSUM") as stp, \
                 tc.tile_pool(name="op_", bufs=1, space="PSUM") as op_, \
                 tc.tile_pool(name="aux", bufs=2, space="PSUM") as aux, \
                 tc.tile_pool(name="ptp", bufs=26) as ptp, \
                 tc.tile_pool(name="osb", bufs=3) as osb, \
                 tc.tile_pool(name="rcpp", bufs=2) as rcpp, \
                 tc.tile_pool(name="stg", bufs=2) as stgp:

                def qk_proj(dst, wt, bcol, mt, s, per_head=False):
                    """dst[:, mt, 256s:+256] = (W^T x^T)[128ch, 256tok] + bias.

                    per_head (for the zero-padded qT): each head's 64
                    channels land in its own head-slot, same partitions."""
                    ps = aux.tile([128, 512], f32, name="ps", tag="aux")
                    for kt in range(KT):
                        nc.tensor.matmul(
                            ps[:, 0:QB],
                            lhsT=wt[:, kt, mt * 128 : (mt + 1) * 128],
                            rhs=xt[:, kt, s * QB : (s + 1) * QB],
                            start=(kt == 0),
                            stop=(kt == KT - 1),
                        )
                    if per_head:
                        for hh in range(2):
                            a = 64 * hh
                            nc.vector.tensor_scalar_add(
                                dst[a : a + 64, 2 * mt + hh, s * QB : (s + 1) * QB],
                                ps[a : a + 64, 0:QB],
                                bqk_sb[a : a + 64, bcol + mt : bcol + mt + 1],
                            )
                    else:
                        nc.vector.tensor_scalar_add(
                            dst[:, mt, s * QB : (s + 1) * QB],
                            ps[:, 0:QB],
                            bqk_sb[:, bcol + mt : bcol + mt + 1],
                        )

                def v_proj(tt):
                    """v_sb[:, tt, :, 0:64] = ([x;1] @ [Wv;bv])[128tok, 256ch]."""
                    ps = aux.tile([128, 8, 64], f32, name="psv", tag="aux")
                    for kt in range(KT):
                        nc.tensor.matmul(
                            ps[:, 0:NH, :],
                            lhsT=xt[:, kt, tt * 128 : (tt + 1) * 128],
                            rhs=wv_sb[:, kt, :],
                            start=(kt == 0),
                            stop=(kt == KT - 1),
                        )
                    nc.vector.tensor_add(
                        v_sb[:, tt, :, 0:64], ps[:, 0:NH, :], wvb_sb[:]
                    )

                def transpose_pair(osb_t, qt, s):
                    """oT_sb[:, blk, qt*256+128s:+128] = osb_t[:, blk, :]^T.

                    DMA crossbar (off the PE/DVE path) in steady state; PE
                    transpose for the last qt where DMA latency would land
                    on the drain tail."""
                    tr = aux.tile([128, 2, 128], bf16, name="tr", tag="aux")
                    for blk in range(2):
                        nc.tensor.transpose(tr[:, blk, :], osb_t[:, 128 * blk : 128 * (blk + 1)], ident_sb[:])
                    for blk in range(2):
                        nc.vector.tensor_copy(
                            oT_sb[:, blk, qt * QB + 128 * s : qt * QB + 128 * (s + 1)],
                            tr[:, blk, :],
                        )

                def outproj(tt):
                    """out[tt*128:+128, :] = sum_ct oT_ct^T @ Wo_ct.

                    Stage copies go through the idle Pool engine in steady
                    state (keeps DVE free for the recip/normalize chain);
                    the last qt alternates DVE/ACT so the tail copies run
                    in parallel."""
                    tail = tt >= 2 * (NQT - 1)
                    stg = None
                    for nt in range(2):
                        ps = aux.tile([128, 512], f32, name="ops", tag="aux")
                        for ct in range(2):
                            nc.tensor.matmul(
                                ps[:],
                                lhsT=oT_sb[:, ct, tt * 128 : (tt + 1) * 128],
                                rhs=wo_sb[:, ct, nt * 512 : (nt + 1) * 512],
                                start=(ct == 0),
                                stop=(ct == 1),
                            )
                        if not tail:
                            stg = stgp.tile([128, 512], bf16, name="stg", tag="stg")
                            nc.vector.tensor_copy(stg[:], ps[:])
                            nc.sync.dma_start(
                                out[
                                    tt * 128 : (tt + 1) * 128,
                                    nt * 512 : (nt + 1) * 512,
                                ],
                                stg[:],
                            )
                            continue
                        # tail: copies run on DVE and ACT in parallel, one
                        # batched store per token tile
                        if nt == 0:
                            stg = stgp.tile([128, 1024], bf16, name="stg2", tag="st2")
                            nc.vector.tensor_copy(stg[:, 0:512], ps[:])
                        else:
                            nc.scalar.copy(stg[:, 512:1024], ps[:])
                            nc.sync.dma_start(
                                out[tt * 128 : (tt + 1) * 128, :], stg[:]
                            )

                def emit(item):
                    kind = item[0]
                    if kind == "q":
                        qk_proj(qT_sb, wq_sb, 0, item[1], item[2])
                    elif kind == "k":
                        qk_proj(kT_sb, wk_sb, 2, item[1], item[2])
                    elif kind == "v":
                        v_proj(item[1])
                    elif kind == "o":
                        outproj(item[1])

                from collections import deque

                pending = deque()

                # ones column of [v|1] (fused softmax denominators)
                nc.vector.memset(v_sb[:, :, :, 64:65], 1.0)
                # zero the off-head partitions of the per-head padded qT so
                # S can contract over all 128 partitions at tile base 0
                # (on the idle gpsimd engine, off the DVE critical path;
                # qt0's columns first so the first S never waits)
                for h in range(NH):
                    a = 64 * ((h + 1) % 2)
                    nc.gpsimd.memset(qT_sb[a : a + 64, h, 0:QB], 0.0)
                for h in range(NH):
                    a = 64 * ((h + 1) % 2)
                    nc.gpsimd.memset(qT_sb[a : a + 64, h, QB:N], 0.0)

                # PE p-state warmup: keep the PE continuously busy from t~0 so
                # it reaches full clock (3us ramp) before the real work lands.
                warm = pp.tile([128, 256], bf16)
                nc.vector.memset(warm[:], 0.0)
                wps = aux.tile([128, 512], f32, name="wps", tag="aux")
                for i in range(11 if WARMUP else 0):
                    nc.tensor.matmul(
                        wps[:, 0:256],
                        lhsT=warm[:, 0:128],
                        rhs=warm[:, :],
                        start=True,
                        stop=True,
                        skip_group_check=True,
                    )

                # front: projections for tokens 0-255 (x slice 0 only), in
                # DMA-arrival order; v goes after the first S/exp so the late
                # wv DMA never blocks the ACT stream start
                qk_proj(kT_sb, wk_sb, 2, 0, 0)
                qk_proj(qT_sb, wq_sb, 0, 0, 0, per_head=True)
                qk_proj(kT_sb, wk_sb, 2, 1, 0)
                qk_proj(qT_sb, wq_sb, 0, 1, 0, per_head=True)

                def s_exp(qt, kt2, split=False):
                    """S^T + exp for (qt, kt2); returns the pt tile.

                    split=True runs exp per head-pair so the first tiles
                    start as soon as the mt0 projections land."""
                    st = stp.tile([128, NH, QB], f32, name="st", tag="st")
                    pt = ptp.tile([128, NH, QB], bf16, name="pt", tag="pt")
                    for half in range(2):
                        for hh in range(2):
                            h = 2 * half + hh
                            nc.tensor.matmul(
                                st[:, h, :],
                                lhsT=kT_sb[
                                    :, h // 2, kt2 * 128 : (kt2 + 1) * 128
                                ],
                                rhs=qT_sb[:, h, qt * QB : (qt + 1) * QB],
                                start=True,  # overwrite; st is read-only after
                                stop=True,
                                skip_group_check=True,
                            )
                        if split and half == 0:
                            nc.scalar.activation(
                                pt[:, 0:2, :], st[:, 0:2, :], Exp, scale=SCALE
                            )
                    if split:
                        nc.scalar.activation(
                            pt[:, 2:4, :], st[:, 2:4, :], Exp, scale=SCALE
                        )
                    else:
                        nc.scalar.activation(pt[:], st[:], Exp, scale=SCALE)
                    return pt

                def pv(pt, kt2, o_ps):
                    first, last = kt2 == 0, kt2 == TOKT - 1
                    for s in range(2):
                        for h in range(NH):
                            nc.tensor.matmul(
                                o_ps[:, s, 65 * h : 65 * h + 65],
                                lhsT=pt[:, h, 128 * s : 128 * (s + 1)],
                                rhs=v_sb[:, kt2, h, :],
                                start=(first and h == 0),  # opener per s-bank
                                stop=last,
                                skip_group_check=True,
                            )

                def norm(qt, o_ps):
                    rcp = rcpp.tile([128, 8], f32, name="rcp", tag="rcp")
                    for s in range(2):
                        nc.vector.reciprocal(
                            rcp[:, 4 * s : 4 * s + 4], o_ps[:, s, 64:324:65]
                        )
                        osb_t = osb.tile([128, 256], bf16, name="osb", tag="osb")
                        for h in range(NH):
                            nc.vector.tensor_scalar_mul(
                                osb_t[:, 64 * h : 64 * h + 64],
                                o_ps[:, s, 65 * h : 65 * h + 64],
                                rcp[:, 4 * s + h : 4 * s + h + 1],
                            )
                        # crossbar transpose issues immediately (SP engine);
                        # the out-proj chains stream through later ACT windows
                        if DO_TAIL:
                            transpose_pair(osb_t, qt, s)
                            if qt == NQT - 1:
                                outproj(2 * qt + s)
                            else:
                                pending.append((BODY[0] + 3, ("o", 2 * qt + s)))

                # Deferred PV/normalize stream: ACT runs ahead of PV by up to
                # LAG tiles (deep pt pool), so the heavy qt0 projection
                # backlog never stalls the exp stream; the lag drains over
                # the last two qts to keep the tail short.
                LAG0 = 20
                HUMP = 3 * TOKT
                END = (NQT - 1) * TOKT + 8
                pv_q = deque()
                BODY = [0]
                pt_next = s_exp(0, 0, split=SPLIT_EXP)
                v_proj(0)
                v_proj(1)
                for qt in range(NQT):
                    o_ps = op_.tile([128, 2, 512], f32, name="o_ps", tag="o")
                    for kt2 in range(TOKT):
                        body = qt * TOKT + kt2
                        BODY[0] = body
                        last = kt2 == TOKT - 1
                        pv_q.append(
                            lambda pt=pt_next, kt2=kt2, o=o_ps: pv(pt, kt2, o)
                        )
                        # software pipeline: S/exp of the NEXT tile go ahead of
                        # this tile's PV so the ACT stream never waits on PE
                        if not last:
                            pt_next = s_exp(qt, kt2 + 1, split=(SPLIT_EXP and qt == 0 and kt2 == 0))
                        elif qt + 1 < NQT:
                            pt_next = s_exp(qt + 1, 0)
                        # high-priority streamed projections; kT is on the
                        # exp critical path, v only feeds the (deeply lagged)
                        # PV stream so it spreads into qt1's idle PE windows
                        if qt == 0:
                            s_next = kt2 // 2 + 1
                            if kt2 % 2 == 0 and s_next < 8:
                                qk_proj(kT_sb, wk_sb, 2, 0, s_next)
                                qk_proj(kT_sb, wk_sb, 2, 1, s_next)
                        if DO_V and 2 <= body - 10 < TOKT:
                            v_proj(body - 10)
                        if kt2 == 8 and qt + 1 < NQT:
                            qk_proj(qT_sb, wq_sb, 0, 0, qt + 1, per_head=True)
                        if kt2 == 9 and qt + 1 < NQT:
                            qk_proj(qT_sb, wq_sb, 0, 1, qt + 1, per_head=True)
                        if kt2 >= 2 and pending and body >= pending[0][0]:
                            emit(pending.popleft()[1])
                            if qt == NQT - 1 and pending and body >= pending[0][0]:
                                emit(pending.popleft()[1])
                        lag = max(14, LAG0 - max(0, body - HUMP) // 2)
                        if body >= END:
                            lag = 1
                        while DO_PV and len(pv_q) > lag:
                            pv_q.popleft()()
                    pv_q.append(lambda qt=qt, o=o_ps: norm(qt, o))
                while DO_PV and pv_q:
                    pv_q.popleft()()
                while pending:
                    emit(pending.popleft()[1])
    nc.finalize()
    return nc


def make_in_maps(x, W_qkv, b_qkv, W_o):
    """Shard full inputs into per-core input maps (core c: batch c//4, group c%4)."""
    import ml_dtypes

    bf16 = ml_dtypes.bfloat16
    x = np.asarray(x, dtype=np.float32)
    W_qkv = np.asarray(W_qkv, dtype=np.float32)
    b_qkv = np.asarray(b_qkv, dtype=np.float32)
    W_o = np.asarray(W_o, dtype=np.float32)

    def pack_w(w):  # [1024, CH] -> [128, KT, CH] partition-major
        return np.ascontiguousarray(
            w.reshape(KT, 128, CH).transpose(1, 0, 2).astype(bf16)
        )

    in_maps = []
    for c in range(NCORES):
        b, g = divmod(c, GROUPS)
        cols = slice(CH * g, CH * (g + 1))
        bq = b_qkv[0 * D : 1 * D][cols]
        bk = b_qkv[1 * D : 2 * D][cols]
        bqk = np.stack(
            [bq[0:128], bq[128:256], bk[0:128], bk[128:256]], axis=1
        ).astype(np.float32)
        xh = (
            x[b].T.reshape(KT, 128, N).transpose(1, 0, 2).astype(bf16)
        )  # [128, KT, N]
        m = {
            "xh": np.ascontiguousarray(xh),
            "wq": pack_w(W_qkv[:, 0 * D : 1 * D][:, cols]),
            "wk": pack_w(W_qkv[:, 1 * D : 2 * D][:, cols]),
            "wv": pack_w(W_qkv[:, 2 * D : 3 * D][:, cols]),
            "wvb": np.ascontiguousarray(
                np.broadcast_to(
                    b_qkv[2 * D : 3 * D][cols].reshape(NH, SUB), (128, NH, SUB)
                ).astype(bf16)
            ),
            "wo": np.ascontiguousarray(
                W_o[cols, :].reshape(2, 128, D).transpose(1, 0, 2).astype(bf16)
            ),
            "bqk": np.ascontiguousarray(bqk),
            "ones": np.ones((128, 128), dtype=bf16),
            "ident": np.eye(128, dtype=bf16),
        }
        in_maps.append(m)
    return in_maps


_NC = None


def get_nc():
    global _NC
    if _NC is None:
        _NC = build_nc()
    return _NC


def kernel(x, W_qkv, b_qkv, W_o, b_o):
    from concourse import bass_utils

    b_o = np.asarray(b_o, dtype=np.float32)
    in_maps = make_in_maps(x, W_qkv, b_qkv, W_o)
    res = bass_utils.run_bass_kernel_spmd(get_nc(), in_maps, core_ids=list(range(NCORES)))
    out = np.empty((B, N, D), dtype=np.float32)
    for b in range(B):
        acc = res.results[4 * b]["out"].astype(np.float32)
        for g in range(1, GROUPS):
            acc += res.results[4 * b + g]["out"].astype(np.float32)
        out[b] = acc + b_o
    return out
